# revision 16
# baseline (speedup 1.0000x reference)
"""Multi-head causal attention (b=4, t=2048, k=1024, h=16) on 8 Trainium2 cores.

Sharding: core c = (batch b=c//2, head-group g=c%2). Each core computes one
batch x 8 heads; the two half-head partial outputs per batch are summed on
host.

Per-core kernel (bf16 matmul paths, fp32 PSUM), fully software-pipelined so
the tensor engine never idles (idle gaps also drop the PE clock 2.4->1.2GHz):
  - Q/K projections write per-head zero-PADDED tiles qtp/ktp[128, h, t]
    (head data in partitions 0:64, zeros in 64:128): the PE runs at half
    rate when contraction or stationary dims are < 128. Odd heads are
    placed via SBUF->SBUF DMA partition shift.
  - V is stored [128, ki, h, 128]: cols 0:64 = V, col 64 = ones (softmax
    denominator via the augmented PV matmul), 65:128 zeros.
  - Emission order interleaves projection matmul groups between attention
    chunks: K0/Q0 first, V tiles just-in-time inside head 0, K/Q tile g+1
    as filler during heads 2g-2..2g-1, output projection (phase C) m-tiles
    as filler inside head 7 as their ot columns complete.
  - Softmax: exp on ACT (bf16 out); reciprocal as exp(-ln d) on ACT (both
    funcs forced into one activation table => a single table load);
    denominator broadcast via a [1,128] ones matmul; normalize on DVE.
"""
import sys

sys.path.insert(0, "/opt/trn_rl_repo")

import numpy as np
import ml_dtypes

import concourse.bass as bass
import concourse.mybir as mybir
import concourse.tile as tile
from concourse import bacc
from concourse.bass_utils import run_bass_kernel_spmd
from concourse.masks import make_upper_triangular

# Force every ACT func (Exp/Ln/Copy) onto the one table that contains them
# all, so the table-load pass emits a single load instead of ping-ponging
# between exp-only and ln-only tables (1.3us per reload). Indices into
# act_info.json are preserved; only the candidacy of the other tables is
# hidden from the chooser.
_ORIG_GET_TABLES = bacc.get_activation_tables


def _single_table_get_activation_tables(arch):
    tabs = _ORIG_GET_TABLES(arch)
    if "natural_log_exp_and_others" not in tabs:
        return tabs
    return {
        name: (funcs if name == "natural_log_exp_and_others" else set())
        for name, funcs in tabs.items()
    }


bacc.get_activation_tables = _single_table_get_activation_tables

F32 = mybir.dt.float32
F32R = mybir.dt.float32r
BF16 = mybir.dt.bfloat16
EXP = mybir.ActivationFunctionType.Exp
LN = mybir.ActivationFunctionType.Ln

B, T, KD, NH, HS = 4, 2048, 1024, 16, 64
NCORES = 8


def build_nc(t=T, dl=512, hl=8, kd=KD):
    """One core's program: x.T [kd,t], per-group weights, partial out [t,kd]."""
    nk = kd // 128       # contraction tiles for projections
    mt = t // 128        # t tiles (also k-position tiles in attention)
    dt = dl // 128       # local-dim tiles
    nqc = t // 512       # q chunks
    scale = 1.0 / float(np.sqrt(kd))

    nc = bacc.Bacc("TRN2", target_bir_lowering=False, debug=False, num_devices=NCORES)
    xt_d = nc.dram_tensor("xt", [kd, t], BF16, kind="ExternalInput")
    wq_d = nc.dram_tensor("wq", [kd, dl], BF16, kind="ExternalInput")
    wk_d = nc.dram_tensor("wk", [kd, dl], BF16, kind="ExternalInput")
    wv_d = nc.dram_tensor("wv", [kd, dl], BF16, kind="ExternalInput")
    wo_d = nc.dram_tensor("wo", [dl, kd], BF16, kind="ExternalInput")
    out_d = nc.dram_tensor("out", [t, kd], F32, kind="ExternalOutput")

    with tile.TileContext(nc) as tc:
        with (
            tc.tile_pool(name="persist", bufs=1) as pp,
            tc.tile_pool(name="misc", bufs=1) as mp,
            tc.tile_pool(name="pa", bufs=1) as pa,
            tc.tile_pool(name="past", bufs=4) as past,
            tc.tile_pool(name="pbe", bufs=4) as pbe,
            tc.tile_pool(name="pbm", bufs=2) as pbm,
            tc.tile_pool(name="pco", bufs=2) as pco,
            tc.tile_pool(name="ps", bufs=1, space="PSUM") as ps,
        ):
            qtp = pp.tile([128, hl, t], BF16)   # per-head padded Q^T
            ktp = pp.tile([128, hl, t], BF16)   # per-head padded K^T
            v_s = pp.tile([128, mt, hl, 128], BF16)  # V | ones | zeros
            ot_s = pp.tile([128, dt, t], BF16)
            wo_s = pp.tile([128, dt, kd], BF16)
            mask_t = mp.tile([128, 128], BF16)
            ones_t = mp.tile([1, 128], BF16)
            ones128 = mp.tile([128, 128], BF16)
            xt_s = pa.tile([128, nk, t], BF16)
            wq_s = pa.tile([128, nk, dl], BF16)
            wk_s = pa.tile([128, nk, dl], BF16)
            wv_s = pa.tile([128, nk, dl], BF16)

            # ---------------- input DMAs ----------------
            xt_r = xt_d[:, :].rearrange("(n p) t -> p n t", p=128)
            for k in range(nk):
                nc.sync.dma_start(xt_s[:, k, :], xt_r[:, k, :])
            nc.sync.dma_start(
                wk_s[:, :, :], wk_d[:, :].rearrange("(n p) d -> p n d", p=128)
            )
            nc.sync.dma_start(
                wq_s[:, :, :], wq_d[:, :].rearrange("(n p) d -> p n d", p=128)
            )
            nc.sync.dma_start(
                wv_s[:, :, :], wv_d[:, :].rearrange("(n p) d -> p n d", p=128)
            )
            nc.sync.dma_start(
                wo_s[:, :, :], wo_d[:, :].rearrange("(n p) o -> p n o", p=128)
            )

            # ---------------- constants + padding zeros ----------------
            make_upper_triangular(nc, mask_t[:, :], val=1.0, diag=True)
            nc.vector.memset(ones128[:, :], 1.0)
            nc.scalar.copy(ones_t[:, :], ones128[0:1, :])
            nc.scalar.copy(
                v_s[:, :, :, 64],
                ones128[:, 0 : mt * hl].rearrange("p (m h) -> p m h", m=mt),
            )
            # fine-grained zeroing in first-use order: head h's pad gates
            # only that head's scores; v pad per ki gates only that PV step.
            nc.gpsimd.memset(qtp[64:128, 0, :], 0.0)
            nc.gpsimd.memset(ktp[64:128, 0, :], 0.0)
            for ki in range(4):
                nc.gpsimd.memset(v_s[:, ki, :, 65:128], 0.0)
            for h in range(1, hl):
                nc.gpsimd.memset(qtp[64:128, h, :], 0.0)
                nc.gpsimd.memset(ktp[64:128, h, :], 0.0)
                if h < 4:
                    for ki in range(4 * h, 4 * h + 4):
                        nc.gpsimd.memset(v_s[:, ki, :, 65:128], 0.0)

            # ---------------- emission helpers ----------------
            def proj_v(m):
                psv = ps.tile([128, dl], F32, name=f"psv{m}", tag="proj", bufs=1)
                for k in range(nk):
                    nc.tensor.matmul(
                        psv[:, :],
                        xt_s[:, k, 128 * m : 128 * m + 128],
                        wv_s[:, k, :],
                        start=(k == 0),
                        stop=(k == nk - 1),
                    )
                nc.vector.tensor_copy(
                    v_s[:, m, :, 0:64],
                    psv[:, :].rearrange("p (h d) -> p h d", h=hl),
                )

            def proj_qk(w_s, o_s, pfx, m, n):
                # one 512-col group of Q or K dtile m (heads 2m, 2m+1)
                cols = slice(512 * n, 512 * n + 512)
                psq = ps.tile(
                    [128, 512], F32, name=f"ps{pfx}{m}_{n}", tag="proj", bufs=1
                )
                for k in range(nk):
                    nc.tensor.matmul(
                        psq[:, :],
                        w_s[:, k, 128 * m : 128 * m + 128],
                        xt_s[:, k, cols],
                        start=(k == 0),
                        stop=(k == nk - 1),
                    )
                nc.vector.tensor_copy(o_s[0:64, 2 * m, cols], psq[0:64, :])
                st = past.tile(
                    [128, 512], BF16, name=f"st{pfx}{m}_{n}", tag="stage"
                )
                nc.vector.tensor_copy(st[64:128, :], psq[64:128, :])
                nc.sync.dma_start(o_s[0:64, 2 * m + 1, cols], st[64:128, :])

            def attn_ki(h, ki, otp):
                q0 = 128 * ki
                for qc in range(q0 // 512, nqc):
                    off = max(q0, 512 * qc)
                    w = 512 * (qc + 1) - off
                    stp = ps.tile(
                        [128, 512], F32, name=f"st{h}_{ki}_{qc}", tag="st", bufs=2
                    )
                    nc.tensor.matmul(
                        stp[:, :w],
                        ktp[:, h, q0 : q0 + 128],
                        qtp[:, h, off : off + w],
                        start=True,
                        stop=True,
                    )
                    ex = pbe.tile(
                        [128, 512], BF16, name=f"ex{h}_{ki}_{qc}", tag="exp"
                    )
                    nc.scalar.activation(ex[:, :w], stp[:, :w], EXP, scale=scale)
                    if off == q0:
                        nc.vector.tensor_mul(ex[:, 0:128], ex[:, 0:128], mask_t[:, :])
                    co = off - 512 * qc
                    nc.tensor.matmul(
                        otp[qc][:, co : co + w],
                        v_s[:, ki, h, :],
                        ex[:, :w],
                        start=(ki == 0),
                        stop=(ki == 4 * qc + 3),
                    )

            def normalize(h, qc, otp):
                mh, ph = h // 2, 64 * (h % 2)
                # 1/d = exp(-ln d) on ACT (no table swap), broadcast across
                # 64 partitions with a ones[1,128] matmul, multiply on DVE.
                rec = pbm.tile([1, 512], F32, name=f"rc{h}_{qc}", tag="rec")
                recb = pbm.tile([1, 512], BF16, name=f"rb{h}_{qc}", tag="recb")
                with nc.allow_low_precision(reason="softmax denom"):
                    nc.scalar.activation(rec[:, :], otp[qc][64:65, :], LN)
                    nc.scalar.activation(recb[:, :], rec[:, :], EXP, scale=-1.0)
                bc = ps.tile([128, 512], F32, name=f"bc{h}_{qc}", tag="bc", bufs=1)
                nc.tensor.matmul(
                    bc[:, :], ones_t[:, :], recb[:, :], start=True, stop=True
                )
                cols = slice(512 * qc, 512 * qc + 512)
                with nc.allow_low_precision(reason="softmax normalize"):
                    if ph == 0:
                        dst = ot_s[0:64, mh, cols]
                        nc.vector.tensor_copy(dst, otp[qc][0:64, :])
                        nc.vector.tensor_mul(dst, dst, bc[0:64, :])
                    else:
                        sc = pbm.tile(
                            [64, 512], BF16, name=f"sc{h}_{qc}", tag="scr"
                        )
                        nc.vector.tensor_copy(sc[:, :], otp[qc][0:64, :])
                        nc.vector.tensor_mul(sc[:, :], sc[:, :], bc[0:64, :])
                        nc.sync.dma_start(ot_s[64:128, mh, cols], sc[:, :])

            def phasec(m):
                ob = pco.tile([128, kd], F32, name=f"ob{m}", tag="ob")
                for n in range(kd // 512):
                    pso = ps.tile(
                        [128, 512], F32, name=f"pso{m}_{n}", tag="proj", bufs=1
                    )
                    for k in range(dt):
                        nc.tensor.matmul(
                            pso[:, :],
                            ot_s[:, k, 128 * m : 128 * m + 128],
                            wo_s[:, k, 512 * n : 512 * n + 512],
                            start=(k == 0),
                            stop=(k == dt - 1),
                        )
                    nc.vector.tensor_copy(ob[:, 512 * n : 512 * n + 512], pso[:, :])
                nc.sync.dma_start(out_d[128 * m : 128 * m + 128, :], ob[:, :])

            # ---------------- emission schedule ----------------
            for n in range(nqc):
                proj_qk(wk_s, ktp, "k", 0, n)
            for n in range(nqc):
                proj_qk(wq_s, qtp, "q", 0, n)

            # filler projection groups: heads 2g,2g+1 need K/Q tile g,
            # emitted during heads 2g-2 and 2g-1
            filler = {h: [] for h in range(hl)}
            for g in range(1, dt):
                ksrc = [
                    (lambda gg=g, nn=n: proj_qk(wk_s, ktp, "k", gg, nn))
                    for n in range(nqc)
                ]
                qsrc = [
                    (lambda gg=g, nn=n: proj_qk(wq_s, qtp, "q", gg, nn))
                    for n in range(nqc)
                ]
                filler[2 * g - 2] += ksrc
                filler[2 * g - 1] += qsrc

            cready = []   # phase C m-tiles ready to emit (during h7)
            for h in range(hl):
                otp = [
                    ps.tile([128, 512], F32, name=f"otp{h}_{qc}", tag="ot", bufs=4)
                    for qc in range(nqc)
                ]
                fill = filler[h]
                fi = 0
                for ki in range(mt):
                    if h == 0:
                        proj_v(ki)
                    attn_ki(h, ki, otp)
                    want = (ki + 1) * len(fill) // mt
                    while fi < want:
                        fill[fi]()
                        fi += 1
                    if ki % 4 == 3:
                        qc = ki // 4
                        normalize(h, qc, otp)
                        if h == hl - 1:
                            cready += range(4 * qc, 4 * qc + 4)
                    if h == hl - 1 and cready:
                        phasec(cready.pop(0))
            while cready:
                phasec(cready.pop(0))

    nc.finalize()
    return nc


_NC_CACHE = {}


def _get_nc(key=(T, 512, 8, KD)):
    if key not in _NC_CACHE:
        _NC_CACHE[key] = build_nc(*key)
    return _NC_CACHE[key]


def make_in_maps(x, Wq, Wk, Wv, Wo, dl=512):
    in_maps = []
    for c in range(NCORES):
        b, g = c // 2, c % 2
        rows = slice(dl * g, dl * (g + 1))
        in_maps.append(
            {
                "xt": np.ascontiguousarray(x[b].T).astype(ml_dtypes.bfloat16),
                "wq": np.ascontiguousarray(Wq[rows, :].T).astype(ml_dtypes.bfloat16),
                "wk": np.ascontiguousarray(Wk[rows, :].T).astype(ml_dtypes.bfloat16),
                "wv": np.ascontiguousarray(Wv[rows, :].T).astype(ml_dtypes.bfloat16),
                "wo": np.ascontiguousarray(Wo[:, rows].T).astype(ml_dtypes.bfloat16),
            }
        )
    return in_maps


def run_spmd(x, Wq, Wk, Wv, Wo, trace=False):
    nc = _get_nc()
    in_maps = make_in_maps(x, Wq, Wk, Wv, Wo)
    res = run_bass_kernel_spmd(nc, in_maps, list(range(NCORES)), trace=trace)
    outs = [res.results[c]["out"] for c in range(NCORES)]
    final = np.stack([outs[2 * b] + outs[2 * b + 1] for b in range(B)])
    return final.astype(np.float32), res


def kernel(x, Wq, Wk, Wv, Wo):
    x = np.asarray(x, dtype=np.float32)
    Wq = np.asarray(Wq, dtype=np.float32)
    Wk = np.asarray(Wk, dtype=np.float32)
    Wv = np.asarray(Wv, dtype=np.float32)
    out, _ = run_spmd(x, Wq, Wk, Wv, np.asarray(Wo, dtype=np.float32))
    return out
